# revision 1
# baseline (speedup 1.0000x reference)
"""Trainium2 Bass kernel for nn_AttnEncoder (attention-gated LSTM encoder).

Math note: in the reference, the softmax attention score is
s[b,d] = (h.wh)[b] + (c.wc)[b] + x_time[b,d] + b_attn, and softmax is taken
over d. The h/c/bias terms are constant along d, so they cancel in softmax:
attn = softmax(x_time) — independent of the recurrence and of t. The model
therefore reduces to an LSTM over w_in_t = attn * x_t with attn computed once.

Layout: everything transposed — features on SBUF partitions, batch on the
free axis. 8-way data parallel over batch (512 batch rows per core).

Per core:
  phase A: DMA x^T tiles [D=128, BC=512] per t; accumulate x_time via DVE STT.
  phase B: softmax over partitions via Exp(ACT) + ones-matmul(PE) column sum +
           reciprocal(DVE) + K=1 broadcast matmul(PE) + multiply.
  phase C: 64 LSTM steps: gates = W_ih.(attn*x_t) + W_hh.h + b in PSUM
           (8 bf16 matmuls), sigmoid/tanh on ACT (fp32, per-gate bias via
           activation bias AP), cell update on DVE in fp32, h in bf16.
"""

import numpy as np
import ml_dtypes

B, T, D, H = 4096, 64, 128, 128
NCORES = 8
BC = B // NCORES          # 512 batch rows per core
G4 = 4 * H                # 512 gate rows
GATE_PERM = [1, 0, 2, 3]  # PSUM gate order [f, i, g, o] from torch [i, f, g, o]

_CACHE = {}


def _legalize_waits(nc, max_waits=1):
    """This container's walrus supports at most one sync wait per instruction.
    Hoist excess waits onto preceding single-wait NoOps on the same engine."""
    import bass_rust

    seq = 0
    for f in nc.m.functions:
        for bb in f.blocks:
            if not any(
                i.sync_info is not None and len(i.sync_info.on_wait) > max_waits
                for i in bb.instructions
            ):
                continue
            new_insts = []
            for inst in bb.instructions:
                si = inst.sync_info
                if si is not None and len(si.on_wait) > max_waits:
                    waits = list(si.on_wait)
                    for w in waits[:-max_waits]:
                        seq += 1
                        nop = bass_rust.InstNoOp(
                            name=f"waitsplit-{seq}", engine=inst.engine
                        )
                        nop.sync_info = bass_rust.SyncInfo(on_wait=[w], on_update=[])
                        new_insts.append(nop)
                    inst.sync_info = bass_rust.SyncInfo(
                        on_wait=waits[-max_waits:], on_update=list(si.on_update)
                    )
                new_insts.append(inst)
            bb.instructions = new_insts


def _build_program():
    import concourse.bass as bass
    import concourse.tile as tile
    from concourse import mybir

    f32 = mybir.dt.float32
    bf16 = mybir.dt.bfloat16
    AF = mybir.ActivationFunctionType
    OP = mybir.AluOpType

    nc = bass.Bass("TRN2", num_devices=NCORES)
    x_d = nc.dram_tensor("x", [T, D, BC], f32, kind="ExternalInput")
    wih_d = nc.dram_tensor("wih", [D, G4], bf16, kind="ExternalInput")
    whh_d = nc.dram_tensor("whh", [H, G4], bf16, kind="ExternalInput")
    bias_d = nc.dram_tensor("bias", [H, 4], f32, kind="ExternalInput")
    wt_d = nc.dram_tensor("wt", [H, T], f32, kind="ExternalInput")
    y_d = nc.dram_tensor("y", [T, H, BC], bf16, kind="ExternalOutput")

    with tile.TileContext(nc) as tc:
        with (
            tc.tile_pool(name="const", bufs=1) as const,
            tc.tile_pool(name="work", bufs=2) as work,
            tc.tile_pool(name="state", bufs=2) as state,
        ):
            wih = const.tile([D, G4], bf16)
            nc.sync.dma_start(out=wih[:], in_=wih_d[:])
            whh = const.tile([H, G4], bf16)
            nc.sync.dma_start(out=whh[:], in_=whh_d[:])
            bias = const.tile([H, 4], f32)
            nc.sync.dma_start(out=bias[:], in_=bias_d[:])
            wtt = const.tile([H, T], f32)
            nc.sync.dma_start(out=wtt[:], in_=wt_d[:])
            onesK = const.tile([128, 1], f32)
            nc.vector.memset(onesK[:], 1.0)
            ones1 = const.tile([1, 128], f32)
            nc.vector.memset(ones1[:], 1.0)

            # resident input, [D, T*BC] fp32 (128 KiB per partition)
            xs = const.tile([D, T * BC], f32)
            for t in range(T):
                nc.sync.dma_start(
                    out=xs[:, t * BC : (t + 1) * BC], in_=x_d[t, :, :]
                )

            # phase A: x_time = sum_t wt[t] * x_t  (ping-pong STT accumulate)
            acc = work.tile([D, BC], f32, tag="acc")
            nc.vector.memset(acc[:], 0.0)
            for t in range(T):
                acc_new = work.tile([D, BC], f32, tag="acc")
                nc.vector.scalar_tensor_tensor(
                    out=acc_new[:],
                    in0=xs[:, t * BC : (t + 1) * BC],
                    scalar=wtt[:, t : t + 1],
                    in1=acc[:],
                    op0=OP.mult,
                    op1=OP.add,
                )
                acc = acc_new

            # phase B: attn = softmax over partition dim of acc
            attn = const.tile([D, BC], bf16)
            e = const.tile([D, BC], f32)
            nc.scalar.activation(out=e[:], in_=acc[:], func=AF.Exp)
            with tc.tile_pool(name="psumB", bufs=1, space="PSUM") as pb:
                s = pb.tile([1, BC], f32, tag="colsum")
                nc.tensor.matmul(s[:], onesK[:], e[:], start=True, stop=True)
                rs = const.tile([1, BC], f32)
                nc.vector.reciprocal(out=rs[:], in_=s[:])
                rb = pb.tile([128, BC], f32, tag="bcast")
                nc.tensor.matmul(rb[:], ones1[:], rs[:], start=True, stop=True)
                nc.vector.tensor_tensor(
                    out=attn[:], in0=e[:], in1=rb[:], op=OP.mult
                )

            # phase C: LSTM recurrence
            h_prev = state.tile([H, BC], bf16, tag="h")
            nc.vector.memset(h_prev[:], 0.0)
            c_prev = state.tile([H, BC], f32, tag="c")
            nc.vector.memset(c_prev[:], 0.0)

            with tc.tile_pool(name="psum", bufs=2, space="PSUM") as psum:
                for t in range(T):
                    ps = psum.tile([128, 4 * BC], f32, tag="gates")
                    w_in = work.tile([D, BC], bf16, tag="win")
                    nc.gpsimd.tensor_tensor(
                        out=w_in[:],
                        in0=attn[:],
                        in1=xs[:, t * BC : (t + 1) * BC],
                        op=OP.mult,
                    )
                    for g in range(4):
                        nc.tensor.matmul(
                            ps[:, g * BC : (g + 1) * BC],
                            wih[:, g * H : (g + 1) * H],
                            w_in[:],
                            start=True,
                            stop=False,
                        )
                    for g in range(4):
                        nc.tensor.matmul(
                            ps[:, g * BC : (g + 1) * BC],
                            whh[:, g * H : (g + 1) * H],
                            h_prev[:],
                            start=False,
                            stop=True,
                        )
                    # gates in PSUM order [f, i, g, o]
                    sf = work.tile([H, BC], f32, tag="sf")
                    nc.scalar.activation(
                        out=sf[:], in_=ps[:, 0:BC], func=AF.Sigmoid,
                        bias=bias[:, 0:1],
                    )
                    si = work.tile([H, BC], f32, tag="si")
                    nc.scalar.activation(
                        out=si[:], in_=ps[:, BC : 2 * BC], func=AF.Sigmoid,
                        bias=bias[:, 1:2],
                    )
                    tg = work.tile([H, BC], f32, tag="tg")
                    nc.scalar.activation(
                        out=tg[:], in_=ps[:, 2 * BC : 3 * BC], func=AF.Tanh,
                        bias=bias[:, 2:3],
                    )
                    so = work.tile([H, BC], f32, tag="so")
                    nc.scalar.activation(
                        out=so[:], in_=ps[:, 3 * BC : 4 * BC], func=AF.Sigmoid,
                        bias=bias[:, 3:4],
                    )
                    m1 = work.tile([H, BC], f32, tag="m1")
                    nc.vector.tensor_tensor(
                        out=m1[:], in0=sf[:], in1=c_prev[:], op=OP.mult
                    )
                    m2 = work.tile([H, BC], f32, tag="m2")
                    nc.vector.tensor_tensor(
                        out=m2[:], in0=si[:], in1=tg[:], op=OP.mult
                    )
                    c_new = state.tile([H, BC], f32, tag="c")
                    nc.vector.tensor_tensor(
                        out=c_new[:], in0=m1[:], in1=m2[:], op=OP.add
                    )
                    tch = work.tile([H, BC], f32, tag="tch")
                    nc.scalar.activation(out=tch[:], in_=c_new[:], func=AF.Tanh)
                    h_new = state.tile([H, BC], bf16, tag="h")
                    nc.vector.tensor_tensor(
                        out=h_new[:], in0=so[:], in1=tch[:], op=OP.mult
                    )
                    nc.sync.dma_start(out=y_d[t, :, :], in_=h_new[:])
                    h_prev, c_prev = h_new, c_new

    _legalize_waits(nc)
    return nc


def _make_runner(nc):
    """jit-once sharded executor modeled on bass2jax.run_bass_via_pjrt."""
    import jax
    import jax.core
    from jax.experimental.shard_map import shard_map
    from jax.sharding import Mesh, PartitionSpec
    from concourse import mybir
    from concourse.bass2jax import (
        _bass_exec_p,
        install_neuronx_cc_hook,
        partition_id_tensor,
    )

    install_neuronx_cc_hook()

    partition_name = nc.partition_id_tensor.name if nc.partition_id_tensor else None
    in_names, out_names, out_avals, zero_outs = [], [], [], []
    for alloc in nc.m.functions[0].allocations:
        if not isinstance(alloc, mybir.MemoryLocationSet):
            continue
        name = alloc.memorylocations[0].name
        if alloc.kind == "ExternalInput":
            if name != partition_name:
                in_names.append(name)
        elif alloc.kind == "ExternalOutput":
            shape = tuple(alloc.tensor_shape)
            dtype = mybir.dt.np(alloc.dtype)
            out_names.append(name)
            out_avals.append(jax.core.ShapedArray(shape, dtype))
            zero_outs.append(np.zeros(shape, dtype))
    n_params = len(in_names)
    n_outs = len(out_avals)
    all_in_names = list(in_names) + list(out_names)
    if partition_name is not None:
        all_in_names.append(partition_name)
    donate = tuple(range(n_params, n_params + n_outs))

    def _body(*args):
        operands = list(args)
        if partition_name is not None:
            operands.append(partition_id_tensor())
        outs = _bass_exec_p.bind(
            *operands,
            out_avals=tuple(out_avals),
            in_names=tuple(all_in_names),
            out_names=tuple(out_names),
            lowering_input_output_aliases=(),
            sim_require_finite=True,
            sim_require_nnan=True,
            nc=nc,
        )
        return tuple(outs)

    devices = jax.devices()[:NCORES]
    mesh = Mesh(np.asarray(devices), ("core",))
    in_specs = (PartitionSpec("core"),) * (n_params + n_outs)
    out_specs = (PartitionSpec("core"),) * n_outs
    sharded = jax.jit(
        shard_map(
            _body, mesh=mesh, in_specs=in_specs, out_specs=out_specs,
            check_rep=False,
        ),
        donate_argnums=donate,
        keep_unused=True,
    )

    def run(per_core_inputs):
        """per_core_inputs: list (len NCORES) of dicts name->np array.
        Returns list of dicts name->np array."""
        concat_in = [
            np.concatenate(
                [np.asarray(per_core_inputs[c][n]) for c in range(NCORES)], axis=0
            )
            for n in in_names
        ]
        concat_zeros = [
            np.zeros((NCORES * z.shape[0], *z.shape[1:]), z.dtype) for z in zero_outs
        ]
        out_arrs = sharded(*concat_in, *concat_zeros)
        return [
            {
                n: np.asarray(out_arrs[i]).reshape(NCORES, *out_avals[i].shape)[c]
                for i, n in enumerate(out_names)
            }
            for c in range(NCORES)
        ]

    run.in_names = in_names
    run.out_names = out_names
    run.out_avals = out_avals
    run.zero_outs = zero_outs
    run.sharded = sharded
    return run


def _get_runner():
    if "runner" not in _CACHE:
        nc = _build_program()
        _CACHE["runner"] = _make_runner(nc)
    return _CACHE["runner"]


def _prep_inputs(input_data, W_ih, W_hh, b_ih, b_hh, W_attn, b_attn):
    input_data = np.ascontiguousarray(np.asarray(input_data, dtype=np.float32))
    W_ih = np.asarray(W_ih, dtype=np.float32)
    W_hh = np.asarray(W_hh, dtype=np.float32)
    b = np.asarray(b_ih, dtype=np.float32) + np.asarray(b_hh, dtype=np.float32)
    W_attn = np.asarray(W_attn, dtype=np.float32)

    wih_r = np.ascontiguousarray(
        W_ih.reshape(4, H, D)[GATE_PERM].reshape(G4, D).T
    ).astype(ml_dtypes.bfloat16)
    whh_r = np.ascontiguousarray(
        W_hh.reshape(4, H, H)[GATE_PERM].reshape(G4, H).T
    ).astype(ml_dtypes.bfloat16)
    bias_r = np.ascontiguousarray(b.reshape(4, H)[GATE_PERM].T)  # [H, 4]
    wt = W_attn[0, 2 * H :]  # [T]
    wt_rep = np.ascontiguousarray(np.broadcast_to(wt[None, :], (H, T)))

    per_core = []
    for c in range(NCORES):
        xc = np.ascontiguousarray(
            input_data[c * BC : (c + 1) * BC].transpose(1, 2, 0)
        )  # [T, D, BC]
        per_core.append(
            {"x": xc, "wih": wih_r, "whh": whh_r, "bias": bias_r, "wt": wt_rep}
        )
    return per_core


def _assemble_output(results):
    out = np.empty((B, T, H), dtype=np.float32)
    for c in range(NCORES):
        yc = results[c]["y"]  # [T, H, BC] bf16
        out[c * BC : (c + 1) * BC] = yc.astype(np.float32).transpose(2, 0, 1)
    return out


def kernel(**inputs):
    per_core = _prep_inputs(**inputs)
    run = _get_runner()
    results = run(per_core)
    return _assemble_output(results)


# revision 2
# speedup vs baseline: 4324.6846x; 4324.6846x over previous
"""Trainium2 Bass kernel for nn_AttnEncoder (attention-gated LSTM encoder).

Math note: in the reference, the softmax attention score is
s[b,d] = (h.wh)[b] + (c.wc)[b] + x_time[b,d] + b_attn, and softmax is taken
over d. The h/c/bias terms are constant along d, so they cancel in softmax:
attn = softmax(x_time) — independent of the recurrence and of t. The model
therefore reduces to an LSTM over w_in_t = attn * x_t with attn computed once.

Layout: everything transposed — features on SBUF partitions, batch on the
free axis. 8-way data parallel over batch (512 batch rows per core).

Per core:
  phase A: DMA x^T tiles [D=128, BC=512] per t; accumulate x_time via DVE STT.
  phase B: softmax over partitions via Exp(ACT) + ones-matmul(PE) column sum +
           reciprocal(DVE) + K=1 broadcast matmul(PE) + multiply.
  phase C: 64 LSTM steps: gates = W_ih.(attn*x_t) + W_hh.h + b in PSUM
           (8 bf16 matmuls), sigmoid/tanh on ACT (fp32, per-gate bias via
           activation bias AP), cell update on DVE in fp32, h in bf16.
"""

import numpy as np
import ml_dtypes

B, T, D, H = 4096, 64, 128, 128
NCORES = 8
BC = B // NCORES          # 512 batch rows per core
G4 = 4 * H                # 512 gate rows
GATE_PERM = [1, 0, 2, 3]  # PSUM gate order [f, i, g, o] from torch [i, f, g, o]

_CACHE = {}


def _legalize_waits(nc, max_waits=1):
    """This container's walrus supports at most one sync wait per instruction.
    Hoist excess waits onto preceding single-wait NoOps on the same engine."""
    import bass_rust

    seq = 0
    for f in nc.m.functions:
        for bb in f.blocks:
            if not any(
                i.sync_info is not None and len(i.sync_info.on_wait) > max_waits
                for i in bb.instructions
            ):
                continue
            new_insts = []
            for inst in bb.instructions:
                si = inst.sync_info
                if si is not None and len(si.on_wait) > max_waits:
                    waits = list(si.on_wait)
                    for w in waits[:-max_waits]:
                        seq += 1
                        nop = bass_rust.InstNoOp(
                            name=f"waitsplit-{seq}", engine=inst.engine
                        )
                        nop.sync_info = bass_rust.SyncInfo(on_wait=[w], on_update=[])
                        new_insts.append(nop)
                    inst.sync_info = bass_rust.SyncInfo(
                        on_wait=waits[-max_waits:], on_update=list(si.on_update)
                    )
                new_insts.append(inst)
            bb.instructions = new_insts


def _build_program():
    import concourse.bass as bass
    import concourse.tile as tile
    from concourse import mybir

    f32 = mybir.dt.float32
    bf16 = mybir.dt.bfloat16
    AF = mybir.ActivationFunctionType
    OP = mybir.AluOpType

    nc = bass.Bass("TRN2", num_devices=NCORES)
    x_d = nc.dram_tensor("x", [T, D, BC], f32, kind="ExternalInput")
    wih_d = nc.dram_tensor("wih", [D, G4], bf16, kind="ExternalInput")
    whh_d = nc.dram_tensor("whh", [H, G4], bf16, kind="ExternalInput")
    bias_d = nc.dram_tensor("bias", [H, 4], f32, kind="ExternalInput")
    wt_d = nc.dram_tensor("wt", [H, T], f32, kind="ExternalInput")
    y_d = nc.dram_tensor("y", [T, H, BC], bf16, kind="ExternalOutput")

    with tile.TileContext(nc) as tc:
        with (
            tc.tile_pool(name="const", bufs=1) as const,
            tc.tile_pool(name="work", bufs=2) as work,
            tc.tile_pool(name="state", bufs=2) as state,
        ):
            wih = const.tile([D, G4], bf16)
            nc.sync.dma_start(out=wih[:], in_=wih_d[:])
            whh = const.tile([H, G4], bf16)
            nc.sync.dma_start(out=whh[:], in_=whh_d[:])
            bias = const.tile([H, 4], f32)
            nc.sync.dma_start(out=bias[:], in_=bias_d[:])
            wtt = const.tile([H, T], f32)
            nc.sync.dma_start(out=wtt[:], in_=wt_d[:])
            onesK = const.tile([128, 1], f32)
            nc.vector.memset(onesK[:], 1.0)
            ones1 = const.tile([1, 128], f32)
            nc.vector.memset(ones1[:], 1.0)

            # resident input, [D, T*BC] fp32 (128 KiB per partition)
            xs = const.tile([D, T * BC], f32)
            for t in range(T):
                nc.sync.dma_start(
                    out=xs[:, t * BC : (t + 1) * BC], in_=x_d[t, :, :]
                )

            # phase A: x_time = sum_t wt[t] * x_t  (ping-pong STT accumulate)
            acc = work.tile([D, BC], f32, tag="acc")
            nc.vector.memset(acc[:], 0.0)
            for t in range(T):
                acc_new = work.tile([D, BC], f32, tag="acc")
                nc.vector.scalar_tensor_tensor(
                    out=acc_new[:],
                    in0=xs[:, t * BC : (t + 1) * BC],
                    scalar=wtt[:, t : t + 1],
                    in1=acc[:],
                    op0=OP.mult,
                    op1=OP.add,
                )
                acc = acc_new

            # phase B: attn = softmax over partition dim of acc
            attn = const.tile([D, BC], bf16)
            e = const.tile([D, BC], f32)
            nc.scalar.activation(out=e[:], in_=acc[:], func=AF.Exp)
            with tc.tile_pool(name="psumB", bufs=1, space="PSUM") as pb:
                s = pb.tile([1, BC], f32, tag="colsum")
                nc.tensor.matmul(s[:], onesK[:], e[:], start=True, stop=True)
                rs = const.tile([1, BC], f32)
                nc.vector.reciprocal(out=rs[:], in_=s[:])
                rb = pb.tile([128, BC], f32, tag="bcast")
                nc.tensor.matmul(rb[:], ones1[:], rs[:], start=True, stop=True)
                nc.vector.tensor_tensor(
                    out=attn[:], in0=e[:], in1=rb[:], op=OP.mult
                )

            # phase C: LSTM recurrence
            h_prev = state.tile([H, BC], bf16, tag="h")
            nc.vector.memset(h_prev[:], 0.0)
            c_prev = state.tile([H, BC], f32, tag="c")
            nc.vector.memset(c_prev[:], 0.0)

            with tc.tile_pool(name="psum", bufs=2, space="PSUM") as psum:
                for t in range(T):
                    ps = psum.tile([128, 4 * BC], f32, tag="gates")
                    w_in = work.tile([D, BC], bf16, tag="win")
                    nc.gpsimd.tensor_tensor(
                        out=w_in[:],
                        in0=attn[:],
                        in1=xs[:, t * BC : (t + 1) * BC],
                        op=OP.mult,
                    )
                    for g in range(4):
                        nc.tensor.matmul(
                            ps[:, g * BC : (g + 1) * BC],
                            wih[:, g * H : (g + 1) * H],
                            w_in[:],
                            start=True,
                            stop=False,
                        )
                    for g in range(4):
                        nc.tensor.matmul(
                            ps[:, g * BC : (g + 1) * BC],
                            whh[:, g * H : (g + 1) * H],
                            h_prev[:],
                            start=False,
                            stop=True,
                        )
                    # gates in PSUM order [f, i, g, o]
                    sf = work.tile([H, BC], f32, tag="sf")
                    nc.scalar.activation(
                        out=sf[:], in_=ps[:, 0:BC], func=AF.Sigmoid,
                        bias=bias[:, 0:1],
                    )
                    si = work.tile([H, BC], f32, tag="si")
                    nc.scalar.activation(
                        out=si[:], in_=ps[:, BC : 2 * BC], func=AF.Sigmoid,
                        bias=bias[:, 1:2],
                    )
                    tg = work.tile([H, BC], f32, tag="tg")
                    nc.scalar.activation(
                        out=tg[:], in_=ps[:, 2 * BC : 3 * BC], func=AF.Tanh,
                        bias=bias[:, 2:3],
                    )
                    so = work.tile([H, BC], f32, tag="so")
                    nc.scalar.activation(
                        out=so[:], in_=ps[:, 3 * BC : 4 * BC], func=AF.Sigmoid,
                        bias=bias[:, 3:4],
                    )
                    m1 = work.tile([H, BC], f32, tag="m1")
                    nc.vector.tensor_tensor(
                        out=m1[:], in0=sf[:], in1=c_prev[:], op=OP.mult
                    )
                    m2 = work.tile([H, BC], f32, tag="m2")
                    nc.vector.tensor_tensor(
                        out=m2[:], in0=si[:], in1=tg[:], op=OP.mult
                    )
                    c_new = state.tile([H, BC], f32, tag="c")
                    nc.vector.tensor_tensor(
                        out=c_new[:], in0=m1[:], in1=m2[:], op=OP.add
                    )
                    tch = work.tile([H, BC], f32, tag="tch")
                    nc.scalar.activation(out=tch[:], in_=c_new[:], func=AF.Tanh)
                    h_new = state.tile([H, BC], bf16, tag="h")
                    nc.vector.tensor_tensor(
                        out=h_new[:], in0=so[:], in1=tch[:], op=OP.mult
                    )
                    nc.sync.dma_start(out=y_d[t, :, :], in_=h_new[:])
                    h_prev, c_prev = h_new, c_new

    _legalize_waits(nc)
    return nc


def _make_runner(nc):
    """jit-once sharded executor modeled on bass2jax.run_bass_via_pjrt."""
    import jax
    import jax.core
    from jax.experimental.shard_map import shard_map
    from jax.sharding import Mesh, PartitionSpec
    from concourse import mybir
    from concourse.bass2jax import (
        _bass_exec_p,
        install_neuronx_cc_hook,
        partition_id_tensor,
    )

    install_neuronx_cc_hook()

    partition_name = nc.partition_id_tensor.name if nc.partition_id_tensor else None
    in_names, out_names, out_avals, zero_outs = [], [], [], []
    for alloc in nc.m.functions[0].allocations:
        if not isinstance(alloc, mybir.MemoryLocationSet):
            continue
        name = alloc.memorylocations[0].name
        if alloc.kind == "ExternalInput":
            if name != partition_name:
                in_names.append(name)
        elif alloc.kind == "ExternalOutput":
            shape = tuple(alloc.tensor_shape)
            dtype = mybir.dt.np(alloc.dtype)
            out_names.append(name)
            out_avals.append(jax.core.ShapedArray(shape, dtype))
            zero_outs.append(np.zeros(shape, dtype))
    n_params = len(in_names)
    n_outs = len(out_avals)
    all_in_names = list(in_names) + list(out_names)
    if partition_name is not None:
        all_in_names.append(partition_name)
    donate = tuple(range(n_params, n_params + n_outs))

    def _body(*args):
        operands = list(args)
        if partition_name is not None:
            operands.append(partition_id_tensor())
        outs = _bass_exec_p.bind(
            *operands,
            out_avals=tuple(out_avals),
            in_names=tuple(all_in_names),
            out_names=tuple(out_names),
            lowering_input_output_aliases=(),
            sim_require_finite=True,
            sim_require_nnan=True,
            nc=nc,
        )
        return tuple(outs)

    devices = jax.devices()[:NCORES]
    mesh = Mesh(np.asarray(devices), ("core",))
    in_specs = (PartitionSpec("core"),) * (n_params + n_outs)
    out_specs = (PartitionSpec("core"),) * n_outs
    sharded = jax.jit(
        shard_map(
            _body, mesh=mesh, in_specs=in_specs, out_specs=out_specs,
            check_rep=False,
        ),
        donate_argnums=donate,
        keep_unused=True,
    )

    def run(per_core_inputs):
        """per_core_inputs: list (len NCORES) of dicts name->np array.
        Returns list of dicts name->np array."""
        concat_in = [
            np.concatenate(
                [np.asarray(per_core_inputs[c][n]) for c in range(NCORES)], axis=0
            )
            for n in in_names
        ]
        concat_zeros = [
            np.zeros((NCORES * z.shape[0], *z.shape[1:]), z.dtype) for z in zero_outs
        ]
        out_arrs = sharded(*concat_in, *concat_zeros)
        return [
            {
                n: np.asarray(out_arrs[i]).reshape(NCORES, *out_avals[i].shape)[c]
                for i, n in enumerate(out_names)
            }
            for c in range(NCORES)
        ]

    def _concat_inputs(per_core_inputs):
        return [
            np.concatenate(
                [np.asarray(per_core_inputs[c][n]) for c in range(NCORES)], axis=0
            )
            for n in in_names
        ]

    def make_chain(k):
        """jit-once executor running the bass program k times back-to-back on
        device, chaining each call's y output into the next call's donated
        output buffer (prevents CSE, amortizes dispatch overhead)."""

        def _chain(*args):
            ins = list(args[:n_params])
            outs = list(args[n_params:])
            for _ in range(k):
                operands = ins + outs
                if partition_name is not None:
                    operands = operands + [partition_id_tensor()]
                outs = list(
                    _bass_exec_p.bind(
                        *operands,
                        out_avals=tuple(out_avals),
                        in_names=tuple(all_in_names),
                        out_names=tuple(out_names),
                        lowering_input_output_aliases=(),
                        sim_require_finite=True,
                        sim_require_nnan=True,
                        nc=nc,
                    )
                )
            return tuple(outs)

        return jax.jit(
            shard_map(
                _chain, mesh=mesh, in_specs=in_specs, out_specs=out_specs,
                check_rep=False,
            ),
            donate_argnums=donate,
            keep_unused=True,
        )

    def device_inputs(per_core_inputs):
        import jax as _jax
        from jax.sharding import NamedSharding

        concat_in = _concat_inputs(per_core_inputs)
        shardings = [NamedSharding(mesh, PartitionSpec("core"))] * n_params
        return [
            _jax.device_put(a, s) for a, s in zip(concat_in, shardings)
        ]

    def fresh_zeros():
        return [
            np.zeros((NCORES * z.shape[0], *z.shape[1:]), z.dtype) for z in zero_outs
        ]

    run.in_names = in_names
    run.out_names = out_names
    run.out_avals = out_avals
    run.zero_outs = zero_outs
    run.sharded = sharded
    run.make_chain = make_chain
    run.device_inputs = device_inputs
    run.fresh_zeros = fresh_zeros
    run.mesh = mesh
    return run


def _get_runner():
    if "runner" not in _CACHE:
        nc = _build_program()
        _CACHE["runner"] = _make_runner(nc)
    return _CACHE["runner"]


def _prep_inputs(input_data, W_ih, W_hh, b_ih, b_hh, W_attn, b_attn):
    input_data = np.ascontiguousarray(np.asarray(input_data, dtype=np.float32))
    W_ih = np.asarray(W_ih, dtype=np.float32)
    W_hh = np.asarray(W_hh, dtype=np.float32)
    b = np.asarray(b_ih, dtype=np.float32) + np.asarray(b_hh, dtype=np.float32)
    W_attn = np.asarray(W_attn, dtype=np.float32)

    wih_r = np.ascontiguousarray(
        W_ih.reshape(4, H, D)[GATE_PERM].reshape(G4, D).T
    ).astype(ml_dtypes.bfloat16)
    whh_r = np.ascontiguousarray(
        W_hh.reshape(4, H, H)[GATE_PERM].reshape(G4, H).T
    ).astype(ml_dtypes.bfloat16)
    bias_r = np.ascontiguousarray(b.reshape(4, H)[GATE_PERM].T)  # [H, 4]
    wt = W_attn[0, 2 * H :]  # [T]
    wt_rep = np.ascontiguousarray(np.broadcast_to(wt[None, :], (H, T)))

    per_core = []
    for c in range(NCORES):
        xc = np.ascontiguousarray(
            input_data[c * BC : (c + 1) * BC].transpose(1, 2, 0)
        )  # [T, D, BC]
        per_core.append(
            {"x": xc, "wih": wih_r, "whh": whh_r, "bias": bias_r, "wt": wt_rep}
        )
    return per_core


def _assemble_output(results):
    out = np.empty((B, T, H), dtype=np.float32)
    for c in range(NCORES):
        yc = results[c]["y"]  # [T, H, BC] bf16
        out[c * BC : (c + 1) * BC] = yc.astype(np.float32).transpose(2, 0, 1)
    return out


def kernel(**inputs):
    per_core = _prep_inputs(**inputs)
    run = _get_runner()
    results = run(per_core)
    return _assemble_output(results)


# revision 15
# speedup vs baseline: 9025.1534x; 2.0869x over previous
"""Trainium2 Bass kernel for nn_AttnEncoder (attention-gated LSTM encoder).

Math note: in the reference, the softmax attention score is
s[b,d] = (h.wh)[b] + (c.wc)[b] + x_time[b,d] + b_attn, and softmax is taken
over d. The h/c/bias terms are constant along d, so they cancel in softmax:
attn = softmax(x_time) — independent of the recurrence and of t. The model
therefore reduces to an LSTM over w_in_t = attn * x_t with attn computed once.

Layout: everything transposed — features on SBUF partitions, batch on the
free axis. 8-way data parallel over batch (512 batch rows per core).

Per core:
  phase A: DMA x^T tiles [D=128, BC=512] per t; accumulate x_time via DVE STT.
  phase B: softmax over partitions via Exp(ACT) + ones-matmul(PE) column sum +
           reciprocal(DVE) + K=1 broadcast matmul(PE) + multiply.
  phase C: 64 LSTM steps: gates = W_ih.(attn*x_t) + W_hh.h + b in PSUM
           (8 bf16 matmuls), sigmoid/tanh on ACT (fp32, per-gate bias via
           activation bias AP), cell update on DVE in fp32, h in bf16.
"""

import numpy as np
import ml_dtypes

B, T, D, H = 4096, 64, 128, 128
NCORES = 8
BC = B // NCORES          # 512 batch rows per core
G4 = 4 * H                # 512 gate rows
GATE_PERM = [1, 0, 2, 3]  # PSUM gate order [f, i, g, o] from torch [i, f, g, o]

_CACHE = {}


def _legalize_waits(nc, max_waits=1):
    """This container's walrus supports at most one sync wait per instruction.
    Hoist excess waits onto preceding single-wait NoOps on the same engine."""
    import bass_rust

    seq = 0
    for f in nc.m.functions:
        for bb in f.blocks:
            if not any(
                i.sync_info is not None and len(i.sync_info.on_wait) > max_waits
                for i in bb.instructions
            ):
                continue
            new_insts = []
            for inst in bb.instructions:
                si = inst.sync_info
                if si is not None and len(si.on_wait) > max_waits:
                    waits = list(si.on_wait)
                    for w in waits[:-max_waits]:
                        seq += 1
                        nop = bass_rust.InstNoOp(
                            name=f"waitsplit-{seq}", engine=inst.engine
                        )
                        nop.sync_info = bass_rust.SyncInfo(on_wait=[w], on_update=[])
                        new_insts.append(nop)
                    inst.sync_info = bass_rust.SyncInfo(
                        on_wait=waits[-max_waits:], on_update=list(si.on_update)
                    )
                new_insts.append(inst)
            bb.instructions = new_insts


def _build_program(repeats=1, steps=T, no_dma_in=False, no_dma_out=False,
                   streams=2):
    import concourse.bass as bass
    import concourse.tile as tile
    from concourse import mybir

    f32 = mybir.dt.float32
    bf16 = mybir.dt.bfloat16
    AF = mybir.ActivationFunctionType
    OP = mybir.AluOpType

    nc = bass.Bass("TRN2", num_devices=NCORES)
    x_d = nc.dram_tensor("x", [T, D, BC], f32, kind="ExternalInput")
    wih_d = nc.dram_tensor("wih", [D, G4], bf16, kind="ExternalInput")
    whh_d = nc.dram_tensor("whh", [H, G4], bf16, kind="ExternalInput")
    bias_d = nc.dram_tensor("bias", [H, 4], f32, kind="ExternalInput")
    wt_d = nc.dram_tensor("wt", [H, T], f32, kind="ExternalInput")
    y_d = nc.dram_tensor("y", [T, H, BC], bf16, kind="ExternalOutput")

    with tile.TileContext(nc) as tc:
        with (
            tc.tile_pool(name="const", bufs=1) as const,
            tc.tile_pool(name="work", bufs=2) as work,
            tc.tile_pool(name="state", bufs=2) as state,
        ):
            wih = const.tile([D, G4], bf16)
            nc.sync.dma_start(out=wih[:], in_=wih_d[:])
            whh = const.tile([H, G4], bf16)
            nc.sync.dma_start(out=whh[:], in_=whh_d[:])
            bias = const.tile([H, 4], f32)
            nc.sync.dma_start(out=bias[:], in_=bias_d[:])
            wtt = const.tile([H, T], f32)
            nc.sync.dma_start(out=wtt[:], in_=wt_d[:])
            onesK = const.tile([128, 1], f32)
            nc.vector.memset(onesK[:], 1.0)
            ones1 = const.tile([1, 128], f32)
            nc.vector.memset(ones1[:], 1.0)

            # resident input, [D, T*BC] fp32 (128 KiB per partition)
            xs = const.tile([D, T * BC], f32)
            for rep in range(repeats):
              if not no_dma_in:
                for t in range(T):
                    nc.sync.dma_start(
                        out=xs[:, t * BC : (t + 1) * BC], in_=x_d[t, :, :]
                    )
              elif rep == 0:
                nc.vector.memset(xs[:, 0:BC], 0.01)

              # phase A: x_time = sum_t wt[t] * x_t  (ping-pong STT accumulate)
              acc = work.tile([D, BC], f32, tag="acc")
              nc.vector.memset(acc[:], 0.0)
              for t in range(T):
                acc_new = work.tile([D, BC], f32, tag="acc")
                nc.vector.scalar_tensor_tensor(
                    out=acc_new[:],
                    in0=xs[:, t * BC : (t + 1) * BC],
                    scalar=wtt[:, t : t + 1],
                    in1=acc[:],
                    op0=OP.mult,
                    op1=OP.add,
                )
                acc = acc_new

              # phase B: attn = softmax over partition dim of acc
              attn = work.tile([D, BC], bf16, tag="attn")
              e = work.tile([D, BC], f32, tag="e")
              nc.scalar.activation(out=e[:], in_=acc[:], func=AF.Exp)
              with tc.tile_pool(name="psumB", bufs=1, space="PSUM") as pb:
                s = pb.tile([1, BC], f32, tag="colsum")
                nc.tensor.matmul(s[:], onesK[:], e[:], start=True, stop=True)
                rs = work.tile([1, BC], f32, tag="rs")
                nc.vector.reciprocal(out=rs[:], in_=s[:])
                rb = pb.tile([128, BC], f32, tag="bcast")
                nc.tensor.matmul(rb[:], ones1[:], rs[:], start=True, stop=True)
                nc.vector.tensor_tensor(
                    out=attn[:], in0=e[:], in1=rb[:], op=OP.mult
                )

              # phase C: LSTM recurrence, `streams` interleaved batch slices
              SW = BC // streams  # stream width
              h_prev, c_prev = [], []
              for s in range(streams):
                  hp = state.tile([H, SW], bf16, tag=f"h{s}")
                  nc.vector.memset(hp[:], 0.0)
                  cp = state.tile([H, SW], f32, tag=f"c{s}")
                  nc.vector.memset(cp[:], 0.0)
                  h_prev.append(hp)
                  c_prev.append(cp)

              import bass_rust as _br

              # PSUM packing: one gate per bank at streams=1; two gates per
              # bank at streams=2 (start=True clears the whole bank, so only
              # the first gate in each bank sets start, and explicit deps
              # keep the clearing matmul first).
              BK = 512  # fp32 elements per PSUM bank
              if streams == 1:
                  goff = [0, BK, 2 * BK, 3 * BK]
                  pswidth = 4 * BK
              else:
                  goff = [0, SW, BK, BK + SW]
                  pswidth = 2 * BK
              with tc.tile_pool(name="psum", bufs=2, space="PSUM") as psum:
                for t in range(steps):
                  for s in range(streams):
                    lo = t * BC + s * SW
                    ps = psum.tile([128, pswidth], f32, tag=f"gates{s}")
                    w_in = work.tile([D, SW], bf16, tag=f"win{s}")
                    nc.gpsimd.tensor_tensor(
                        out=w_in[:],
                        in0=attn[:, s * SW : (s + 1) * SW],
                        in1=xs[:, lo : lo + SW],
                        op=OP.mult,
                    )
                    ih_mms = []
                    for g in range(4):
                        mm = nc.tensor.matmul(
                            ps[:, goff[g] : goff[g] + SW],
                            wih[:, g * H : (g + 1) * H],
                            w_in[:],
                            start=(goff[g] % BK == 0),
                            stop=False,
                        )
                        ih_mms.append(mm)
                    if streams > 1:
                        # non-clearing gate must follow its bank's clearer
                        _br.add_dep_helper(
                            ih_mms[1].ins, ih_mms[0].ins, sync=False,
                            reason="bank0 clear order",
                        )
                        _br.add_dep_helper(
                            ih_mms[3].ins, ih_mms[2].ins, sync=False,
                            reason="bank1 clear order",
                        )
                    for g in range(4):
                        nc.tensor.matmul(
                            ps[:, goff[g] : goff[g] + SW],
                            whh[:, g * H : (g + 1) * H],
                            h_prev[s][:],
                            start=False,
                            stop=True,
                        )
                    # gates in PSUM order [f, i, g, o]
                    sf = work.tile([H, SW], f32, tag=f"sf{s}")
                    nc.scalar.activation(
                        out=sf[:], in_=ps[:, goff[0] : goff[0] + SW], func=AF.Sigmoid,
                        bias=bias[:, 0:1],
                    )
                    si = work.tile([H, SW], f32, tag=f"si{s}")
                    nc.scalar.activation(
                        out=si[:], in_=ps[:, goff[1] : goff[1] + SW], func=AF.Sigmoid,
                        bias=bias[:, 1:2],
                    )
                    tg = work.tile([H, SW], f32, tag=f"tg{s}")
                    nc.scalar.activation(
                        out=tg[:], in_=ps[:, goff[2] : goff[2] + SW], func=AF.Tanh,
                        bias=bias[:, 2:3],
                    )
                    so = work.tile([H, SW], f32, tag=f"so{s}")
                    nc.scalar.activation(
                        out=so[:], in_=ps[:, goff[3] : goff[3] + SW], func=AF.Sigmoid,
                        bias=bias[:, 3:4],
                    )
                    m1 = work.tile([H, SW], f32, tag=f"m1{s}")
                    nc.vector.tensor_tensor(
                        out=m1[:], in0=sf[:], in1=c_prev[s][:], op=OP.mult
                    )
                    m2 = work.tile([H, SW], f32, tag=f"m2{s}")
                    nc.vector.tensor_tensor(
                        out=m2[:], in0=si[:], in1=tg[:], op=OP.mult
                    )
                    c_new = state.tile([H, SW], f32, tag=f"c{s}")
                    nc.vector.tensor_tensor(
                        out=c_new[:], in0=m1[:], in1=m2[:], op=OP.add
                    )
                    tch = work.tile([H, SW], f32, tag=f"tch{s}")
                    nc.scalar.activation(out=tch[:], in_=c_new[:], func=AF.Tanh)
                    h_new = state.tile([H, SW], bf16, tag=f"h{s}")
                    nc.vector.tensor_tensor(
                        out=h_new[:], in0=so[:], in1=tch[:], op=OP.mult
                    )
                    if not no_dma_out:
                        nc.sync.dma_start(
                            out=y_d[t, :, s * SW : (s + 1) * SW], in_=h_new[:]
                        )
                    h_prev[s], c_prev[s] = h_new, c_new

    _legalize_waits(nc)
    return nc


def _make_runner(nc):
    """jit-once sharded executor modeled on bass2jax.run_bass_via_pjrt."""
    import jax
    import jax.core
    from jax.experimental.shard_map import shard_map
    from jax.sharding import Mesh, PartitionSpec
    from concourse import mybir
    from concourse.bass2jax import (
        _bass_exec_p,
        install_neuronx_cc_hook,
        partition_id_tensor,
    )

    install_neuronx_cc_hook()

    partition_name = nc.partition_id_tensor.name if nc.partition_id_tensor else None
    in_names, out_names, out_avals, zero_outs = [], [], [], []
    for alloc in nc.m.functions[0].allocations:
        if not isinstance(alloc, mybir.MemoryLocationSet):
            continue
        name = alloc.memorylocations[0].name
        if alloc.kind == "ExternalInput":
            if name != partition_name:
                in_names.append(name)
        elif alloc.kind == "ExternalOutput":
            shape = tuple(alloc.tensor_shape)
            dtype = mybir.dt.np(alloc.dtype)
            out_names.append(name)
            out_avals.append(jax.core.ShapedArray(shape, dtype))
            zero_outs.append(np.zeros(shape, dtype))
    n_params = len(in_names)
    n_outs = len(out_avals)
    all_in_names = list(in_names) + list(out_names)
    if partition_name is not None:
        all_in_names.append(partition_name)
    donate = tuple(range(n_params, n_params + n_outs))

    def _body(*args):
        operands = list(args)
        if partition_name is not None:
            operands.append(partition_id_tensor())
        outs = _bass_exec_p.bind(
            *operands,
            out_avals=tuple(out_avals),
            in_names=tuple(all_in_names),
            out_names=tuple(out_names),
            lowering_input_output_aliases=(),
            sim_require_finite=True,
            sim_require_nnan=True,
            nc=nc,
        )
        return tuple(outs)

    devices = jax.devices()[:NCORES]
    mesh = Mesh(np.asarray(devices), ("core",))
    in_specs = (PartitionSpec("core"),) * (n_params + n_outs)
    out_specs = (PartitionSpec("core"),) * n_outs
    sharded = jax.jit(
        shard_map(
            _body, mesh=mesh, in_specs=in_specs, out_specs=out_specs,
            check_rep=False,
        ),
        donate_argnums=donate,
        keep_unused=True,
    )

    def run(per_core_inputs):
        """per_core_inputs: list (len NCORES) of dicts name->np array.
        Returns list of dicts name->np array."""
        concat_in = [
            np.concatenate(
                [np.asarray(per_core_inputs[c][n]) for c in range(NCORES)], axis=0
            )
            for n in in_names
        ]
        concat_zeros = [
            np.zeros((NCORES * z.shape[0], *z.shape[1:]), z.dtype) for z in zero_outs
        ]
        out_arrs = sharded(*concat_in, *concat_zeros)
        return [
            {
                n: np.asarray(out_arrs[i]).reshape(NCORES, *out_avals[i].shape)[c]
                for i, n in enumerate(out_names)
            }
            for c in range(NCORES)
        ]

    def _concat_inputs(per_core_inputs):
        return [
            np.concatenate(
                [np.asarray(per_core_inputs[c][n]) for c in range(NCORES)], axis=0
            )
            for n in in_names
        ]

    def make_chain(k):
        """jit-once executor running the bass program k times back-to-back on
        device, chaining each call's y output into the next call's donated
        output buffer (prevents CSE, amortizes dispatch overhead)."""

        def _chain(*args):
            ins = list(args[:n_params])
            outs = list(args[n_params:])
            for _ in range(k):
                operands = ins + outs
                if partition_name is not None:
                    operands = operands + [partition_id_tensor()]
                outs = list(
                    _bass_exec_p.bind(
                        *operands,
                        out_avals=tuple(out_avals),
                        in_names=tuple(all_in_names),
                        out_names=tuple(out_names),
                        lowering_input_output_aliases=(),
                        sim_require_finite=True,
                        sim_require_nnan=True,
                        nc=nc,
                    )
                )
            return tuple(outs)

        return jax.jit(
            shard_map(
                _chain, mesh=mesh, in_specs=in_specs, out_specs=out_specs,
                check_rep=False,
            ),
            donate_argnums=donate,
            keep_unused=True,
        )

    def device_inputs(per_core_inputs):
        import jax as _jax
        from jax.sharding import NamedSharding

        concat_in = _concat_inputs(per_core_inputs)
        shardings = [NamedSharding(mesh, PartitionSpec("core"))] * n_params
        return [
            _jax.device_put(a, s) for a, s in zip(concat_in, shardings)
        ]

    def fresh_zeros():
        return [
            np.zeros((NCORES * z.shape[0], *z.shape[1:]), z.dtype) for z in zero_outs
        ]

    run.in_names = in_names
    run.out_names = out_names
    run.out_avals = out_avals
    run.zero_outs = zero_outs
    run.sharded = sharded
    run.make_chain = make_chain
    run.device_inputs = device_inputs
    run.fresh_zeros = fresh_zeros
    run.mesh = mesh
    return run


def _get_runner():
    if "runner" not in _CACHE:
        nc = _build_program()
        _CACHE["runner"] = _make_runner(nc)
    return _CACHE["runner"]


def _prep_inputs(input_data, W_ih, W_hh, b_ih, b_hh, W_attn, b_attn):
    input_data = np.ascontiguousarray(np.asarray(input_data, dtype=np.float32))
    W_ih = np.asarray(W_ih, dtype=np.float32)
    W_hh = np.asarray(W_hh, dtype=np.float32)
    b = np.asarray(b_ih, dtype=np.float32) + np.asarray(b_hh, dtype=np.float32)
    W_attn = np.asarray(W_attn, dtype=np.float32)

    wih_r = np.ascontiguousarray(
        W_ih.reshape(4, H, D)[GATE_PERM].reshape(G4, D).T
    ).astype(ml_dtypes.bfloat16)
    whh_r = np.ascontiguousarray(
        W_hh.reshape(4, H, H)[GATE_PERM].reshape(G4, H).T
    ).astype(ml_dtypes.bfloat16)
    bias_r = np.ascontiguousarray(b.reshape(4, H)[GATE_PERM].T)  # [H, 4]
    wt = W_attn[0, 2 * H :]  # [T]
    wt_rep = np.ascontiguousarray(np.broadcast_to(wt[None, :], (H, T)))

    per_core = []
    for c in range(NCORES):
        xc = np.ascontiguousarray(
            input_data[c * BC : (c + 1) * BC].transpose(1, 2, 0)
        )  # [T, D, BC]
        per_core.append(
            {"x": xc, "wih": wih_r, "whh": whh_r, "bias": bias_r, "wt": wt_rep}
        )
    return per_core


def _assemble_output(results):
    out = np.empty((B, T, H), dtype=np.float32)
    for c in range(NCORES):
        yc = results[c]["y"]  # [T, H, BC] bf16
        out[c * BC : (c + 1) * BC] = yc.astype(np.float32).transpose(2, 0, 1)
    return out


def kernel(**inputs):
    per_core = _prep_inputs(**inputs)
    run = _get_runner()
    results = run(per_core)
    return _assemble_output(results)
